# revision 1
# baseline (speedup 1.0000x reference)
"""Bahdanau additive-attention kernel for Trainium2 (Bass/Tile), 8-core SPMD.

Problem shapes (hardcoded): B=8, S_ENC=256, S_DEC=128, D_ENC=D_DEC=512, UNITS=512.
Sharding: data-parallel over batch B -> one batch element per NeuronCore;
weights replicated.

Math per batch element:
    d_enc = enc @ W_enc + b_enc                    # [256, 512]
    d_dec = dec @ W_dec + b_dec                    # [128, 512]
    scores[q,e] = sum_u tanh(d_dec[q,u] + d_enc[e,u]) * w_score[u]
    weights = softmax(scores, axis=e)              # bias_score cancels in softmax
    out[q,:] = weights[q,:] @ enc                  # [128, 512]

The [128,256,512] tanh intermediate never touches HBM: it is produced in
bf16 SBUF tiles (DVE tensor_scalar broadcast-add at 4x + one large ACT Tanh
per q-block) and consumed immediately by PE as the matmul stationary operand
(bf16 -> fast weight load) against w_score, accumulating scores^T in PSUM.

n_iters > 1 wraps the whole pipeline in a hardware For_i loop (body emitted
once, so program size is independent of n_iters); used only for
wall-clock-delta timing in test.py.
"""

from contextlib import nullcontext

import numpy as np

import concourse.bass as bass
import concourse.tile as tile
from concourse import bacc, mybir
from concourse.bass_utils import run_bass_kernel_spmd
from concourse.masks import make_identity

F32 = mybir.dt.float32
BF16 = mybir.dt.bfloat16
AF = mybir.ActivationFunctionType

S_ENC, S_DEC, D, U = 256, 128, 512, 512
UC = U // 128      # 4 u-chunks
EC = S_ENC // 128  # 2 e-chunks
DC = D // 128      # 4 d-chunks
QB = 8             # q rows per main-loop block
RAMP = [1, 2, 4]   # leading block sizes (pipeline fill)
TAILR = []         # trailing block sizes (pipeline drain)
WBF16 = True       # ship W_enc/W_dec as bf16 (halves weight DMA)
FLIP = False       # score reduction: w-as-stationary waves (True) vs
                   # tanh-as-stationary per-q matmuls (False)
NBLK = S_DEC // QB

N_CORES = 8


def build_program(n_iters: int = 1, qb: int = QB, wbf16: bool = WBF16,
                  blk_bufs: int = 3, gp_adds: int = 0, flip: bool = FLIP,
                  hyb: bool = False):
    """Build the single-core program; SPMD-replicated across 8 cores."""
    nblk = S_DEC // qb
    wdt = BF16 if wbf16 else F32
    nc = bacc.Bacc("TRN2", target_bir_lowering=False, debug=False,
                   num_devices=N_CORES)

    enc_d = nc.dram_tensor("enc", [S_ENC, D], F32, kind="ExternalInput")
    dec_d = nc.dram_tensor("dec", [S_DEC, D], F32, kind="ExternalInput")
    wenc_d = nc.dram_tensor("w_enc", [D, U], wdt, kind="ExternalInput")
    wdec_d = nc.dram_tensor("w_dec", [D, U], wdt, kind="ExternalInput")
    wsc_d = nc.dram_tensor("w_score", [U, 1], F32, kind="ExternalInput")
    benc_d = nc.dram_tensor("b_enc", [U, 1], F32, kind="ExternalInput")
    bdec_d = nc.dram_tensor("b_dec", [U, 1], F32, kind="ExternalInput")
    out_d = nc.dram_tensor("out", [S_DEC, D], F32, kind="ExternalOutput")

    nb = 1 if n_iters == 1 else 2

    with tile.TileContext(nc) as tc:
        with (
            tc.tile_pool(name="const", bufs=1) as constp,
            tc.tile_pool(name="inbuf", bufs=nb) as inp,
            tc.tile_pool(name="proj", bufs=nb) as projp,
            tc.tile_pool(name="args", bufs=blk_bufs) as argsp,
            tc.tile_pool(name="tanh", bufs=blk_bufs) as tanhp,
            tc.tile_pool(name="post", bufs=nb) as postp,
            tc.tile_pool(name="ps_work", bufs=3, space="PSUM") as ps_work,
            tc.tile_pool(name="ps_sc", bufs=1, space="PSUM") as ps_scp,
        ):
            # ---- constants (outside the timing loop: tiny) --------------
            ident = constp.tile([128, 128], F32)
            make_identity(nc, ident[:])
            wsc_f32 = constp.tile([128, UC], F32)        # [u%128, uc]
            nc.sync.dma_start(
                wsc_f32[:], wsc_d.rearrange("(c p) one -> p (c one)", p=128))
            wsc_bf = constp.tile([128, UC], BF16)
            nc.vector.tensor_copy(wsc_bf[:], wsc_f32[:])
            benc_sb = constp.tile([128, UC], F32)
            nc.sync.dma_start(
                benc_sb[:], benc_d.rearrange("(c p) one -> p (c one)", p=128))
            bdec_sb = constp.tile([128, UC], F32)
            nc.sync.dma_start(
                bdec_sb[:], bdec_d.rearrange("(c p) one -> p (c one)", p=128))
            # bias folding: tanh(denc+benc + ddec+bdec) -> denc_raw + (ddec+bsum)
            bsum_sb = constp.tile([128, UC], F32)
            nc.vector.tensor_add(bsum_sb[:], benc_sb[:], bdec_sb[:])

            loop_cm = (tc.For_i(0, n_iters, 1,
                                hint_engines=(mybir.EngineType.PE,
                                              mybir.EngineType.DVE))
                       if n_iters > 1 else nullcontext())
            with loop_cm:
                # ---- input DMAs -----------------------------------------
                # enc/dec land first (transposes need them immediately);
                # weights follow on the same queues (needed ~6us later).
                # Queue spread: sync + scalar HWDGE, gpsimd SWDGE.
                dec_nat = inp.tile([128, D], F32, tag="dec_nat")
                nc.sync.dma_start(dec_nat[:], dec_d[:])
                enc_nat = inp.tile([128, EC * D], F32, tag="enc_nat")
                for ec in range(EC):
                    (nc.gpsimd if ec else nc.scalar).dma_start(
                        enc_nat[:, ec * D:(ec + 1) * D],
                        enc_d[ec * 128:(ec + 1) * 128, :])
                wenc_sb = inp.tile([128, DC * U], wdt, tag="wenc_sb")
                wdec_sb = inp.tile([128, DC * U], wdt, tag="wdec_sb")
                dma_engs = [nc.sync, nc.scalar, nc.gpsimd, nc.sync]
                for dc in range(DC):
                    dma_engs[dc % 4].dma_start(
                        wdec_sb[:, dc * U:(dc + 1) * U],
                        wdec_d[dc * 128:(dc + 1) * 128, :])
                    dma_engs[(dc + 1) % 4].dma_start(
                        wenc_sb[:, dc * U:(dc + 1) * U],
                        wenc_d[dc * 128:(dc + 1) * 128, :])

                # ---- transposes: enc_T[d,(dc x e)], dec_T[d,(dc x q)] ---
                enc_t = inp.tile([128, DC * S_ENC], wdt, tag="enc_t")
                for dc in range(DC):
                    for ec in range(EC):
                        pst = ps_work.tile([128, 128], F32, tag="ps_work",
                                           name="pst")
                        nc.tensor.transpose(
                            pst[:],
                            enc_nat[:, ec * D + dc * 128: ec * D + dc * 128 + 128],
                            ident[:])
                        nc.vector.tensor_copy(
                            enc_t[:, dc * S_ENC + ec * 128:
                                  dc * S_ENC + ec * 128 + 128],
                            pst[:])
                dec_t = inp.tile([128, DC * 128], wdt, tag="dec_t")
                for dc in range(DC):
                    pst = ps_work.tile([128, 128], F32, tag="ps_work",
                                       name="pst")
                    nc.tensor.transpose(
                        pst[:], dec_nat[:, dc * 128:(dc + 1) * 128], ident[:])
                    nc.vector.tensor_copy(
                        dec_t[:, dc * 128:(dc + 1) * 128], pst[:])

                # ---- projections -> transposed, per-uc tiles so the
                # main loop's adds start as soon as each chunk lands -------
                denc_t4 = [projp.tile([128, S_ENC], BF16, tag=f"denc{uc}",
                                      name=f"denc{uc}") for uc in range(UC)]
                ddec_t4 = [projp.tile([128, S_DEC], F32, tag=f"ddec{uc}",
                                      name=f"ddec{uc}") for uc in range(UC)]
                for uc in range(UC):
                    psq = ps_work.tile([128, S_DEC], F32, tag="ps_work",
                                       name="psq")
                    for dc in range(DC):
                        nc.tensor.matmul(
                            psq[:],
                            lhsT=wdec_sb[:, dc * U + uc * 128:
                                         dc * U + uc * 128 + 128],
                            rhs=dec_t[:, dc * 128:(dc + 1) * 128],
                            start=(dc == 0), stop=(dc == DC - 1))
                    nc.vector.tensor_scalar_add(
                        ddec_t4[uc][:], psq[:], bsum_sb[:, uc:uc + 1])
                    psp = ps_work.tile([128, S_ENC], F32, tag="ps_work",
                                       name="psp")
                    for dc in range(DC):
                        nc.tensor.matmul(
                            psp[:],
                            lhsT=wenc_sb[:, dc * U + uc * 128:
                                         dc * U + uc * 128 + 128],
                            rhs=enc_t[:, dc * S_ENC:(dc + 1) * S_ENC],
                            start=(dc == 0), stop=(dc == DC - 1))
                    nc.vector.tensor_copy(denc_t4[uc][:], psp[:])

                # ---- main loop: tanh 4D block + score reduction ---------
                # Scores via w_score-as-stationary (1-column ldweights),
                # tanh tiles as the moving operand; each matmul emits a
                # [1, 512] row of scores (2 q x 256 e) into a PSUM wave,
                # accumulated over the 4 u-chunks. A 1-lane DVE copy plus
                # an SBUF->SBUF DMA scatter lands them as scores[q, e].
                if hyb:
                    # Hybrid: e-chunk 0 via tanh-as-stationary (PE), e-chunk 1
                    # via w-as-stationary waves (PE engine + DVE extract).
                    # Balances PE-seq decode vs DVE so ACT stays saturated.
                    scores_sb = postp.tile([128, S_ENC], F32,
                                           tag="scores_sb")
                    sct0 = ps_scp.tile([128, S_DEC], F32, tag="sct0",
                                       name="sct0")
                    pend = None

                    def emit_scores_h(blk, th):
                        th_r = th[:].rearrange("p (ql uc e) -> p ql uc e",
                                               ql=qb, uc=UC)
                        # old-style: ec=0
                        for ql in range(qb):
                            q = blk * qb + ql
                            for uc in range(UC):
                                nc.tensor.matmul(
                                    sct0[:, q:q + 1],
                                    lhsT=th[:, (ql * UC + uc) * S_ENC:
                                            (ql * UC + uc) * S_ENC + 128],
                                    rhs=wsc_bf[:, uc:uc + 1],
                                    start=(uc == 0), stop=(uc == UC - 1))
                        # flip-style: ec=1 -> wave [1, qb*128]
                        wave = ps_scp.tile([1, qb * 128], F32, tag="wave",
                                           name="wave")
                        for s in range(qb * 128 // 512):
                            for uc in range(UC):
                                nc.tensor.matmul(
                                    wave[0:1, s * 512:(s + 1) * 512],
                                    lhsT=wsc_bf[:, uc:uc + 1],
                                    rhs=th_r[:, 4 * s:4 * s + 4, uc, 128:256],
                                    start=(uc == 0), stop=(uc == UC - 1))
                        wave_sb = postp.tile([1, qb * 128], F32,
                                             tag="wave_sb", name="wave_sb")
                        nc.vector.tensor_copy(wave_sb[:], wave[:])
                        nc.sync.dma_start(
                            scores_sb[blk * qb:(blk + 1) * qb, 128:256],
                            wave_sb[0:1, :])

                    for blk in range(nblk):
                        args = argsp.tile([128, qb * UC * S_ENC], BF16,
                                          tag="args")
                        for ql in range(qb):
                            q = blk * qb + ql
                            for uc in range(UC):
                                nc.vector.tensor_scalar_add(
                                    args[:, (ql * UC + uc) * S_ENC:
                                         (ql * UC + uc + 1) * S_ENC],
                                    denc_t4[uc][:],
                                    ddec_t4[uc][:, q:q + 1])
                        th = tanhp.tile([128, qb * UC * S_ENC], BF16,
                                        tag="th")
                        nc.scalar.activation(th[:], args[:], AF.Tanh)
                        if pend is not None:
                            emit_scores_h(*pend)
                        pend = (blk, th)
                    emit_scores_h(*pend)
                    # assemble ec=0: transpose sct0 [e0,q] -> scores[:, 0:128]
                    sct_sb = postp.tile([128, 128], F32, tag="sct_sb")
                    nc.vector.tensor_copy(sct_sb[:], sct0[:])
                    sc_ps0 = ps_work.tile([128, 128], F32, tag="ps_work",
                                          name="sc_ps0")
                    nc.tensor.transpose(sc_ps0[:], sct_sb[:], ident[:])
                    nc.vector.tensor_copy(scores_sb[:, 0:128], sc_ps0[:])
                elif not flip:
                    sct = [ps_scp.tile([128, S_DEC], F32, tag=f"sct{ec}",
                                       name=f"sct{ec}")
                           for ec in range(EC)]
                    # ramp the first blocks so the first tanh issues after
                    # ~1us of adds instead of the full block's 4us
                    sched = []
                    q0 = 0
                    tail_n = sum(TAILR)
                    for cnt in RAMP + [qb] * S_DEC:
                        cnt = min(cnt, S_DEC - tail_n - q0)
                        if cnt <= 0:
                            break
                        sched.append((q0, cnt))
                        q0 += cnt
                    for cnt in TAILR:
                        sched.append((q0, cnt))
                        q0 += cnt
                    assert q0 == S_DEC
                    for (qs, cnt) in sched:
                        args = argsp.tile([128, cnt * UC * S_ENC], BF16,
                                          tag="args")
                        for ql in range(cnt):
                            q = qs + ql
                            for uc in range(UC):
                                nc.vector.tensor_scalar_add(
                                    args[:, (ql * UC + uc) * S_ENC:
                                         (ql * UC + uc + 1) * S_ENC],
                                    denc_t4[uc][:],
                                    ddec_t4[uc][:, q:q + 1])
                        th = tanhp.tile([128, cnt * UC * S_ENC], BF16,
                                        tag="th")
                        nc.scalar.activation(th[:], args[:], AF.Tanh)
                        for ql in range(cnt):
                            q = qs + ql
                            for ec in range(EC):
                                for uc in range(UC):
                                    nc.tensor.matmul(
                                        sct[ec][:, q:q + 1],
                                        lhsT=th[:, (ql * UC + uc) * S_ENC
                                                + ec * 128:
                                                (ql * UC + uc) * S_ENC
                                                + ec * 128 + 128],
                                        rhs=wsc_bf[:, uc:uc + 1],
                                        start=(uc == 0), stop=(uc == UC - 1))
                    sct_sb = postp.tile([128, S_ENC], F32, tag="sct_sb")
                    for ec in range(EC):
                        nc.vector.tensor_copy(
                            sct_sb[:, ec * 128:(ec + 1) * 128], sct[ec][:])
                    scores_sb = ps_work.tile([128, S_ENC], F32, tag="ps_work",
                                             name="sc_ps")
                    for ec in range(EC):
                        nc.tensor.transpose(
                            scores_sb[:, ec * 128:(ec + 1) * 128],
                            sct_sb[:, ec * 128:(ec + 1) * 128], ident[:])
                else:
                    scores_sb = postp.tile([128, S_ENC], F32,
                                           tag="scores_sb")
                    pend = None  # (blk, th tile) awaiting score reduction

                    def emit_scores(blk, th):
                        th_r = th[:].rearrange("p (ql uc e) -> p ql uc e",
                                               ql=qb, uc=UC)
                        # PSUM waves of <= 2048 f32 (4 banks); 512-wide f-slices
                        # (2 q each) accumulated over the 4 u-chunks.
                        qpw = min(qb, 8)             # q rows per wave
                        for w in range(qb // qpw):
                            wave = ps_scp.tile([1, qpw * S_ENC], F32, tag="wave",
                                               name="wave")
                            for s in range(qpw // 2):
                                for uc in range(UC):
                                    nc.tensor.matmul(
                                        wave[0:1, s * 512:(s + 1) * 512],
                                        lhsT=wsc_bf[:, uc:uc + 1],
                                        rhs=th_r[:, w * qpw + 2 * s:
                                                 w * qpw + 2 * s + 2, uc, :],
                                        start=(uc == 0), stop=(uc == UC - 1))
                            wave_sb = postp.tile([1, qpw * S_ENC], F32,
                                                 tag="wave_sb", name="wave_sb")
                            nc.vector.tensor_copy(wave_sb[:], wave[:])
                            nc.sync.dma_start(
                                scores_sb[blk * qb + w * qpw:
                                          blk * qb + (w + 1) * qpw, :],
                                wave_sb[0:1, :])

                    for blk in range(nblk):
                        args = argsp.tile([128, qb * UC * S_ENC], BF16, tag="args")
                        for ql in range(qb):
                            q = blk * qb + ql
                            for uc in range(UC):
                                eng = (nc.gpsimd if (ql * UC + uc) < gp_adds
                                       else nc.vector)
                                eng.tensor_scalar_add(
                                    args[:, (ql * UC + uc) * S_ENC:
                                         (ql * UC + uc + 1) * S_ENC],
                                    denc_t4[uc][:],
                                    ddec_t4[uc][:, q:q + 1])
                        th = tanhp.tile([128, qb * UC * S_ENC], BF16, tag="th")
                        nc.scalar.activation(th[:], args[:], AF.Tanh)
                        if pend is not None:
                            emit_scores(*pend)
                        pend = (blk, th)
                    emit_scores(*pend)

                # ---- softmax over e -------------------------------------
                neg_max = postp.tile([128, 1], F32, tag="neg_max")
                nc.vector.tensor_reduce(
                    neg_max[:], scores_sb[:], axis=mybir.AxisListType.X,
                    op=mybir.AluOpType.max, negate=True)
                exp_sb = postp.tile([128, S_ENC], F32, tag="exp_sb")
                nc.scalar.activation(exp_sb[:], scores_sb[:], AF.Exp,
                                     bias=neg_max[:, 0:1])
                ssum = postp.tile([128, 1], F32, tag="ssum")
                nc.vector.tensor_reduce(
                    ssum[:], exp_sb[:], axis=mybir.AxisListType.X,
                    op=mybir.AluOpType.add)
                srec = postp.tile([128, 1], F32, tag="srec")
                nc.vector.reciprocal(srec[:], ssum[:])
                wts = postp.tile([128, S_ENC], F32, tag="wts")
                nc.vector.tensor_scalar_mul(wts[:], exp_sb[:], srec[:, 0:1])

                # ---- context = weights @ enc ----------------------------
                wts_t = postp.tile([128, S_ENC], F32, tag="wts_t")
                for ec in range(EC):
                    pst2 = ps_work.tile([128, 128], F32, tag="ps_work",
                                        name="pst2")
                    nc.tensor.transpose(
                        pst2[:], wts[:, ec * 128:(ec + 1) * 128], ident[:])
                    nc.vector.tensor_copy(
                        wts_t[:, ec * 128:(ec + 1) * 128], pst2[:])
                ctx_ps = ps_work.tile([128, D], F32, tag="ps_work",
                                      name="ctx_ps")
                for ec in range(EC):
                    nc.tensor.matmul(
                        ctx_ps[:],
                        lhsT=wts_t[:, ec * 128:(ec + 1) * 128],
                        rhs=enc_nat[:, ec * D:(ec + 1) * D],
                        start=(ec == 0), stop=(ec == EC - 1))
                out_sb = postp.tile([128, D], F32, tag="out_sb")
                nc.scalar.activation(out_sb[:], ctx_ps[:], AF.Copy)
                nc.sync.dma_start(out_d[:], out_sb[:])

    nc.compile()
    return nc


_CACHED = {}


def _get_program(n_iters: int = 1, qb: int = QB, wbf16: bool = WBF16,
                 flip: bool = FLIP, hyb: bool = False):
    key = (n_iters, qb, wbf16, flip, hyb)
    if key not in _CACHED:
        _CACHED[key] = build_program(n_iters, qb, wbf16, flip=flip, hyb=hyb)
    return _CACHED[key]


def _make_in_maps(encodings, decodings, W_enc, W_dec, W_score,
                  bias_enc, bias_dec, wbf16=WBF16):
    wdt = np.dtype("bfloat16") if False else None
    if wbf16:
        import ml_dtypes
        wnp = ml_dtypes.bfloat16
    else:
        wnp = np.float32
    enc = np.ascontiguousarray(np.asarray(encodings, dtype=np.float32))
    dec = np.ascontiguousarray(np.asarray(decodings, dtype=np.float32))
    com = {
        "w_enc": np.ascontiguousarray(np.asarray(W_enc).astype(wnp)),
        "w_dec": np.ascontiguousarray(np.asarray(W_dec).astype(wnp)),
        "w_score": np.asarray(W_score, dtype=np.float32).reshape(U, 1),
        "b_enc": np.asarray(bias_enc, dtype=np.float32).reshape(U, 1),
        "b_dec": np.asarray(bias_dec, dtype=np.float32).reshape(U, 1),
    }
    return [{"enc": enc[i], "dec": dec[i], **com} for i in range(N_CORES)]


_RUNNERS = {}


def _get_runner(key, nc):
    """Persistent jitted executor for nc (run_bass_via_pjrt rebuilds the
    jax.jit on every call; this caches it so repeat calls skip retracing)."""
    if key in _RUNNERS:
        return _RUNNERS[key]

    import jax
    from jax.experimental.shard_map import shard_map
    from jax.sharding import Mesh, PartitionSpec
    from concourse import bass2jax, mybir as mb

    bass2jax.install_neuronx_cc_hook()
    assert nc.dbg_addr is None
    part_name = (nc.partition_id_tensor.name
                 if nc.partition_id_tensor else None)

    in_names, out_names, out_avals = [], [], []
    for alloc in nc.m.functions[0].allocations:
        if not isinstance(alloc, mb.MemoryLocationSet):
            continue
        name = alloc.memorylocations[0].name
        if alloc.kind == "ExternalInput":
            if name != part_name:
                in_names.append(name)
        elif alloc.kind == "ExternalOutput":
            out_avals.append(jax.core.ShapedArray(
                tuple(alloc.tensor_shape), mb.dt.np(alloc.dtype)))
            out_names.append(name)
    n_params = len(in_names)
    all_names = in_names + out_names + ([part_name] if part_name else [])
    donate = tuple(range(n_params, n_params + len(out_names)))

    def _body(*args):
        operands = list(args)
        if part_name is not None:
            operands.append(bass2jax.partition_id_tensor())
        outs = bass2jax._bass_exec_p.bind(
            *operands, out_avals=tuple(out_avals), in_names=tuple(all_names),
            out_names=tuple(out_names), lowering_input_output_aliases=(),
            sim_require_finite=True, sim_require_nnan=True, nc=nc)
        return tuple(outs)

    devices = jax.devices()[:N_CORES]
    mesh = Mesh(np.asarray(devices), ("core",))
    # Per-core inputs are concatenated along axis 0 and core-sharded; the
    # (identical) weights/biases are passed once and replicated by shard_map.
    sharded_names = {"enc", "dec"}
    in_specs = tuple(
        PartitionSpec("core") if n in sharded_names else PartitionSpec()
        for n in in_names) + (PartitionSpec("core"),) * len(out_names)
    sharded = jax.jit(
        shard_map(_body, mesh=mesh, in_specs=in_specs,
                  out_specs=(PartitionSpec("core"),) * len(out_names),
                  check_rep=False),
        donate_argnums=donate, keep_unused=True)

    def runner(in_maps):
        concat_in = [
            np.concatenate([np.asarray(m[name]) for m in in_maps], axis=0)
            if name in sharded_names else np.asarray(in_maps[0][name])
            for name in in_names]
        concat_zeros = [
            np.zeros((N_CORES * a.shape[0], *a.shape[1:]), a.dtype)
            for a in out_avals]
        out_arrs = sharded(*concat_in, *concat_zeros)
        return [
            {name: np.asarray(out_arrs[i]).reshape(
                N_CORES, *out_avals[i].shape)[c]
             for i, name in enumerate(out_names)}
            for c in range(N_CORES)]

    _RUNNERS[key] = runner
    return runner


def run(n_iters=1, qb=QB, wbf16=WBF16, flip=FLIP, hyb=False, **inputs):
    nc = _get_program(n_iters, qb, wbf16, flip, hyb)
    in_maps = _make_in_maps(
        inputs["encodings"], inputs["decodings"], inputs["W_enc"],
        inputs["W_dec"], inputs["W_score"], inputs["bias_enc"],
        inputs["bias_dec"], wbf16)
    results = _get_runner((n_iters, qb, wbf16, flip, hyb), nc)(in_maps)
    return np.stack([results[i]["out"] for i in range(N_CORES)], axis=0)


def kernel(encodings, decodings, W_enc, W_dec, W_score,
           bias_enc, bias_dec, bias_score):
    # bias_score shifts all scores equally and cancels in the softmax.
    del bias_score
    return run(1, encodings=encodings, decodings=decodings, W_enc=W_enc,
               W_dec=W_dec, W_score=W_score, bias_enc=bias_enc,
               bias_dec=bias_dec)

